# revision 5
# baseline (speedup 1.0000x reference)
"""GCN AutoEncoder (6-layer, BN+ReLU) on 8 Trainium2 NeuronCores.

Strategy (dst-sharded graph parallel):
  - nodes partitioned contiguously: core i owns rows [i*NPC, (i+1)*NPC)
  - per layer: local transform t = dis * (bn_fold(v) @ W)  (node-major tiles)
    -> AllGather t -> h_all table in DRAM
    -> per 128-edge chunk: indirect-DMA gather rows h_all[src], build
       selection matrix S[e,d] = (dst_rel[e]==d) on DVE, PE matmul
       M.T @ S accumulated in PSUM per 128-dst tile (feature-major agg)
    -> self-loop contribution via PE transpose of local t tile (no gather)
    -> epilogue: v = relu(dis*agg + b); BN folded into next W via stats
       AllReduce (mean/var -> scale rows of W, bias row c)
Edges (+padding to 128-multiples per dst-tile, equalized across cores so
the SPMD instruction stream is identical) are preprocessed on the host.
"""
import os
import sys

sys.path.insert(0, "/opt/trn_rl_repo")

import numpy as np

import concourse.bass as bass
import concourse.mybir as mybir
import concourse.tile as tile
from concourse import bacc
from concourse.bass_utils import run_bass_kernel_spmd

F32 = mybir.dt.float32
BF16 = mybir.dt.bfloat16
I32 = mybir.dt.int32
AF = mybir.ActivationFunctionType
ALU = mybir.AluOpType

NCORES = 8
P = 128


class Cfg:
    def __init__(self, n_nodes=50000, dims=None):
        self.n = n_nodes
        self.dims = dims or [(88, 70), (70, 60), (60, 50), (50, 60), (60, 70), (70, 88)]
        # relu / bn applied after layer l's bias
        self.relu = [True, True, False, True, True, False]
        self.bn = [True, True, False, True, True, False]
        self.npc = self.n // NCORES
        assert self.npc * NCORES == self.n
        self.ntiles = (self.npc + P - 1) // P
        self.m_last = self.npc - (self.ntiles - 1) * P
        self.eps = 1e-5


def preprocess(cfg, x, edge_index):
    """Host-side: degrees/dis, dst-sharded per-tile chunked edge arrays."""
    n, npc, ntiles = cfg.n, cfg.npc, cfg.ntiles
    src = np.asarray(edge_index[0], dtype=np.int64).astype(np.int32)
    dst = np.asarray(edge_index[1], dtype=np.int64).astype(np.int32)
    deg = np.bincount(dst, minlength=n).astype(np.float32) + 1.0  # + self loop
    dis = 1.0 / np.sqrt(deg)

    core_of = dst // npc
    tile_of = (dst % npc) // P
    # counts per (core, tile)
    counts = np.zeros((NCORES, ntiles), dtype=np.int64)
    np.add.at(counts, (core_of, tile_of), 1)
    cts = np.maximum(1, np.ceil(counts.max(axis=0) / P).astype(np.int64))  # chunks per tile
    tot_chunks = int(cts.sum())

    # order edges by (core, tile); compute per-edge slot
    order = np.lexsort((tile_of, core_of))
    src_s, dst_s = src[order], dst[order]
    core_s, tile_s = core_of[order], tile_of[order]

    # chunk start offset (in chunks) for each tile
    tile_chunk_start = np.zeros(ntiles, dtype=np.int64)
    tile_chunk_start[1:] = np.cumsum(cts)[:-1]

    src_off = np.zeros((NCORES, P, tot_chunks), dtype=np.int32)
    dst_rel = np.full((NCORES, P, tot_chunks), 200.0, dtype=np.float32)

    # per (core,tile) positions
    pos_in_group = np.zeros(len(src_s), dtype=np.int64)
    grp = core_s * ntiles + tile_s
    # stable running index within each group
    sort_idx = np.argsort(grp, kind="stable")
    gsorted = grp[sort_idx]
    first = np.r_[True, gsorted[1:] != gsorted[:-1]]
    grp_start = np.flatnonzero(first)
    run_idx = np.arange(len(gsorted)) - np.repeat(grp_start, np.diff(np.r_[grp_start, len(gsorted)]))
    pos_in_group[sort_idx] = run_idx

    chunk_idx = tile_chunk_start[tile_s] + pos_in_group // P
    part_idx = pos_in_group % P
    # table row in AG output: blocks of ntiles*P rows per owner core
    src_off[core_s, part_idx, chunk_idx] = (
        (src_s // npc) * (ntiles * P) + (src_s % npc))
    dst_rel[core_s, part_idx, chunk_idx] = (dst_s % npc - tile_s * P).astype(np.float32)

    # per-core node-major aux
    xs = np.asarray(x, dtype=np.float32)
    f_in0 = xs.shape[1]
    xT = np.zeros((NCORES, f_in0, cfg.ntiles * P), dtype=np.float32)
    dis_col = np.zeros((NCORES, P, ntiles), dtype=np.float32)
    fmax = max(fo for _, fo in cfg.dims)
    dis_rep = np.zeros((NCORES, fmax, cfg.ntiles * P), dtype=np.float32)
    for i in range(NCORES):
        sl = slice(i * npc, (i + 1) * npc)
        xT[i, :, :npc] = xs[sl].T
        d = dis[sl]
        dis_col[i, : len(d) - (ntiles - 1) * P, ntiles - 1] = d[(ntiles - 1) * P:]
        for t in range(ntiles - 1):
            dis_col[i, :, t] = d[t * P:(t + 1) * P]
        dis_rep[i, :, :npc] = d[None, :]

    iota = np.tile(np.arange(P, dtype=np.float32), (P, 1))
    ident = np.eye(P, dtype=np.float32)

    return dict(
        src_off=src_off, dst_rel=dst_rel, cts=[int(c) for c in cts],
        xT=xT, dis_col=dis_col, dis_rep=dis_rep, iota=iota, ident=ident,
    )


def build_nc(cfg, cts):
    n, npc, ntiles, m_last = cfg.n, cfg.npc, cfg.ntiles, cfg.m_last
    dims = cfg.dims
    tot_chunks = sum(cts)
    fmax = max(fo for _, fo in dims)
    f_in0 = dims[0][0]
    rg = [list(range(NCORES))]

    nc = bacc.Bacc("TRN2", target_bir_lowering=False, debug=False, num_devices=NCORES)

    # ---- external IO
    xT_e = nc.dram_tensor("xT", [f_in0, ntiles * P], F32, kind="ExternalInput")
    src_off_e = nc.dram_tensor("src_off", [P, tot_chunks], I32, kind="ExternalInput")
    dst_rel_e = nc.dram_tensor("dst_rel", [P, tot_chunks], F32, kind="ExternalInput")
    iota_e = nc.dram_tensor("iota", [P, P], F32, kind="ExternalInput")
    ident_e = nc.dram_tensor("ident", [P, P], F32, kind="ExternalInput")
    dis_col_e = nc.dram_tensor("dis_col", [P, ntiles], F32, kind="ExternalInput")
    dis_rep_e = nc.dram_tensor("dis_rep", [fmax, ntiles * P], F32, kind="ExternalInput")
    b6_rep_e = nc.dram_tensor("b6_rep", [P, dims[5][1]], F32, kind="ExternalInput")
    w_e, b_e, g_e, be_e = [], [], [], []
    for l, (fi, fo) in enumerate(dims):
        w_e.append(nc.dram_tensor(f"W{l}", [fi, fo], F32, kind="ExternalInput"))
        b_e.append(nc.dram_tensor(f"b{l}", [fo, 1], F32, kind="ExternalInput"))
        if cfg.bn[l]:
            g_e.append(nc.dram_tensor(f"g{l}", [fo, 1], F32, kind="ExternalInput"))
            be_e.append(nc.dram_tensor(f"be{l}", [fo, 1], F32, kind="ExternalInput"))
        else:
            g_e.append(None)
            be_e.append(None)
    out_e = nc.dram_tensor("out", [ntiles * P, dims[5][1]], F32, kind="ExternalOutput")

    with tile.TileContext(nc) as tc:
        with (
            tc.tile_pool(name="const", bufs=1) as cpool,
            tc.tile_pool(name="vt", bufs=2) as vtpool,
            tc.tile_pool(name="tsb", bufs=2) as tpool,
            tc.tile_pool(name="mg", bufs=12) as mpool,
            tc.tile_pool(name="ssb", bufs=4) as spool,
            tc.tile_pool(name="eps", bufs=4) as epool,
            tc.tile_pool(name="stat", bufs=2) as stpool,
            tc.tile_pool(name="psA", bufs=3, space="PSUM") as psA,
            tc.tile_pool(name="psB", bufs=3, space="PSUM") as psB,
            tc.tile_pool(name="psC", bufs=2, space="PSUM") as psC,
            tc.tile_pool(name="dram", bufs=1, space="DRAM") as dram,
        ):
            # ---- load constants to SBUF
            def load(pool, e, shape, dtype=F32):
                t = pool.tile(shape, dtype, name=f"c_{e.name}")
                nc.sync.dma_start(t[:], e[:])
                return t

            xT_sb = load(cpool, xT_e, [f_in0, ntiles * P])
            src_sb = load(cpool, src_off_e, [P, tot_chunks], I32)
            drel_sb = load(cpool, dst_rel_e, [P, tot_chunks])
            iota_sb = load(cpool, iota_e, [P, P])
            ident_sb = load(cpool, ident_e, [P, P])  # f32 (layer-5 unused now)
            identb_sb = cpool.tile([P, P], BF16, name="identb")
            nc.vector.tensor_copy(identb_sb[:], ident_sb[:])
            dcol_sb = load(cpool, dis_col_e, [P, ntiles])
            drep_sb = load(cpool, dis_rep_e, [fmax, ntiles * P])
            b6r_sb = load(cpool, b6_rep_e, [P, dims[5][1]])
            w_sb = [load(cpool, w_e[l], [dims[l][0], dims[l][1]]) for l in range(6)]
            b_sb = [load(cpool, b_e[l], [dims[l][1], 1]) for l in range(6)]
            g_sb = [load(cpool, g_e[l], [dims[l][1], 1]) if cfg.bn[l] else None for l in range(6)]
            be_sb = [load(cpool, be_e[l], [dims[l][1], 1]) if cfg.bn[l] else None for l in range(6)]

            # DRAM comm buffers
            ag_in = [dram.tile([ntiles * P, dims[l][1]], BF16, tag=f"agin{l}", name=f"agin{l}") for l in range(6)]
            ag_out = [dram.tile([NCORES * ntiles * P, dims[l][1]], BF16, tag=f"agout{l}", name=f"agout{l}") for l in range(6)]
            ar_in = [dram.tile([dims[l][1], 2], F32, tag=f"arin{l}", name=f"arin{l}") if cfg.bn[l] else None for l in range(6)]
            ar_out = [dram.tile([dims[l][1], 2], F32, tag=f"arout{l}", name=f"arout{l}") if cfg.bn[l] else None for l in range(6)]

            prev_vT = None       # [F_in, ntiles*P] post-activation (pre-bn) values
            bn_cur = None        # (gs, cv) per-partition affine for pending bn apply

            for l in range(6):
                f_in, f_out = dims[l]
                tile_ms = [P] * (ntiles - 1) + [m_last]

                # ---------- transform phase: t = dis * (bn(v) @ W)
                # bn applied in feature-major as per-partition affine on ACT
                t_sb = tpool.tile([P, ntiles * f_out], BF16, tag="tsb", name="tsb")
                for t in range(ntiles):
                    m = tile_ms[t]
                    lhsT = (xT_sb if l == 0 else prev_vT)[:f_in, t * P:t * P + m]
                    if bn_cur is not None:
                        gs_c, cv_c = bn_cur
                        vbn = epool.tile([fmax, P], F32, tag="vbn", name="vbn")
                        nc.scalar.activation(vbn[:f_in, :m], lhsT, AF.Identity,
                                             bias=cv_c[:f_in, 0:1],
                                             scale=gs_c[:f_in, 0:1])
                        lhsT = vbn[:f_in, :m]
                    tps = psB.tile([P, f_out], F32, tag="tps", name="tps")
                    nc.tensor.matmul(tps[:m, :], lhsT=lhsT, rhs=w_sb[l][:f_in, :f_out],
                                     start=True, stop=True)
                    tsl = t_sb[:m, t * f_out:(t + 1) * f_out]
                    nc.vector.tensor_scalar_mul(tsl, tps[:m, :], dcol_sb[:m, t:t + 1])
                nc.sync.dma_start(
                    ag_in[l][:].rearrange("(t p) f -> p t f", p=P),
                    t_sb[:].rearrange("p (t f) -> p t f", f=f_out))

                # ---------- AllGather
                nc.gpsimd.collective_compute(
                    "AllGather", ALU.bypass,
                    ins=[ag_in[l][:].opt()],
                    outs=[ag_out[l][:].opt()],
                    replica_groups=rg,
                )

                # ---------- aggregation phase
                if cfg.bn[l]:
                    ssum = stpool.tile([f_out, ntiles], F32, tag="ssum", name="ssum")
                    ssq = stpool.tile([f_out, ntiles], F32, tag="ssq", name="ssq")
                if l < 5:
                    vT = vtpool.tile([f_out, ntiles * P], F32, tag="vt", name="vt")
                else:
                    out_full = tpool.tile([P, ntiles * dims[5][1]], F32, tag="ofull", name="ofull")

                ctmax = max(cts)
                gc = 0
                for t in range(ntiles):
                    m = tile_ms[t]
                    ct = cts[t]
                    tsl = t_sb[:m, t * f_out:(t + 1) * f_out]
                    if l < 5:
                        # self-loop: transpose local t tile -> [f_out, m]
                        selfT = psC.tile([f_out, P], BF16, tag="selfT", name="selfT")
                        nc.tensor.transpose(selfT[:f_out, :m], tsl, identb_sb[:m, :m])
                        agg = psA.tile([f_out, P], F32, tag="agg", name="agg")
                    else:
                        agg = psA.tile([P, f_out], F32, tag="agg", name="agg")
                    # one batched indirect gather for all chunks of this tile
                    mg = mpool.tile([P, ctmax * f_out], BF16, tag="mg", name="mg")
                    nc.gpsimd.indirect_dma_start(
                        out=mg[:, : ct * f_out], out_offset=None, in_=ag_out[l][:],
                        in_offset=bass.IndirectOffsetOnAxis(
                            ap=src_sb[:, gc:gc + ct], axis=0))
                    for c in range(ct):
                        mgc = mg[:, c * f_out:(c + 1) * f_out]
                        s = spool.tile([P, P], BF16, tag="ssb", name="ssb")
                        nc.vector.tensor_scalar(
                            out=s[:], in0=iota_sb[:], scalar1=drel_sb[:, gc:gc + 1],
                            scalar2=None, op0=ALU.is_equal)
                        if l < 5:
                            nc.tensor.matmul(agg[:f_out, :m], lhsT=mgc,
                                             rhs=s[:, :m], start=(c == 0),
                                             stop=(c == ct - 1))
                        else:
                            nc.tensor.matmul(agg[:m, :f_out], lhsT=s[:, :m],
                                             rhs=mgc, start=(c == 0),
                                             stop=(c == ct - 1))
                        gc += 1

                    if l < 5:
                        # epilogue: v = act(dis * (agg + selfT) + b)
                        # (DVE can read only one PSUM operand -> stage selfT)
                        selfT_sb = epool.tile([f_out, P], F32, tag="eself", name="eself")
                        nc.vector.tensor_copy(selfT_sb[:f_out, :m], selfT[:f_out, :m])
                        tmp = epool.tile([f_out, P], F32, tag="etmp", name="etmp")
                        nc.vector.tensor_tensor(out=tmp[:f_out, :m], in0=agg[:f_out, :m],
                                                in1=selfT_sb[:f_out, :m], op=ALU.add)
                        tmp2 = epool.tile([f_out, P], F32, tag="etmp2", name="etmp2")
                        nc.vector.tensor_tensor(
                            out=tmp2[:f_out, :m], in0=tmp[:f_out, :m],
                            in1=drep_sb[:f_out, t * P:t * P + m], op=ALU.mult)
                        vsl = vT[:f_out, t * P:t * P + m]
                        nc.scalar.activation(
                            vsl, tmp2[:f_out, :m],
                            AF.Relu if cfg.relu[l] else AF.Identity,
                            bias=b_sb[l][:f_out, 0:1])
                        if cfg.bn[l]:
                            nc.vector.tensor_reduce(
                                out=ssum[:f_out, t:t + 1], in_=vsl,
                                axis=mybir.AxisListType.X, op=ALU.add)
                            sq = epool.tile([f_out, P], F32, tag="esq", name="esq")
                            nc.vector.tensor_tensor(out=sq[:f_out, :m], in0=vsl,
                                                    in1=vsl, op=ALU.mult)
                            nc.vector.tensor_reduce(
                                out=ssq[:f_out, t:t + 1], in_=sq[:f_out, :m],
                                axis=mybir.AxisListType.X, op=ALU.add)
                    else:
                        # final: out = dis*(agg + t_self) + b6
                        tmp = epool.tile([P, f_out], F32, tag="ftmp", name="ftmp")
                        nc.vector.tensor_tensor(out=tmp[:m, :], in0=agg[:m, :f_out],
                                                in1=tsl, op=ALU.add)
                        tmp2 = epool.tile([P, f_out], F32, tag="ftmp2", name="ftmp2")
                        nc.vector.tensor_scalar_mul(tmp2[:m, :], tmp[:m, :],
                                                    dcol_sb[:m, t:t + 1])
                        osl = out_full[:m, t * f_out:(t + 1) * f_out]
                        nc.vector.tensor_tensor(out=osl, in0=tmp2[:m, :],
                                                in1=b6r_sb[:m, :f_out], op=ALU.add)

                if l == 5:
                    nc.sync.dma_start(
                        out_e[:].rearrange("(t p) f -> p t f", p=P),
                        out_full[:].rearrange("p (t f) -> p t f", f=dims[5][1]))

                # ---------- stats AllReduce + fold into next W
                if l < 5:
                    if cfg.bn[l]:
                        f_next = dims[l + 1][1]
                        pack = stpool.tile([f_out, 2], F32, tag="pack", name="pack")
                        nc.vector.tensor_reduce(out=pack[:f_out, 0:1],
                                                in_=ssum[:f_out, :ntiles],
                                                axis=mybir.AxisListType.X, op=ALU.add)
                        nc.vector.tensor_reduce(out=pack[:f_out, 1:2],
                                                in_=ssq[:f_out, :ntiles],
                                                axis=mybir.AxisListType.X, op=ALU.add)
                        nc.sync.dma_start(ar_in[l][:], pack[:f_out, :])
                        nc.gpsimd.collective_compute(
                            "AllReduce", ALU.add,
                            ins=[ar_in[l][:].opt()],
                            outs=[ar_out[l][:].opt()],
                            replica_groups=rg,
                        )
                        st = stpool.tile([f_out, 2], F32, tag="st", name="st")
                        nc.sync.dma_start(st[:f_out, :], ar_out[l][:])
                        mu = stpool.tile([f_out, 1], F32, tag="mu", name="mu")
                        nc.vector.tensor_scalar_mul(mu[:f_out, :], st[:f_out, 0:1], 1.0 / cfg.n)
                        msq = stpool.tile([f_out, 1], F32, tag="msq", name="msq")
                        nc.vector.tensor_scalar_mul(msq[:f_out, :], st[:f_out, 1:2], 1.0 / cfg.n)
                        var = stpool.tile([f_out, 1], F32, tag="var", name="var")
                        nc.vector.tensor_tensor(out=var[:f_out, :], in0=mu[:f_out, :],
                                                in1=mu[:f_out, :], op=ALU.mult)
                        nc.vector.tensor_tensor(out=var[:f_out, :], in0=msq[:f_out, :],
                                                in1=var[:f_out, :], op=ALU.subtract)
                        nc.vector.tensor_scalar_add(var[:f_out, :], var[:f_out, :], cfg.eps)
                        rv = stpool.tile([f_out, 1], F32, tag="rv", name="rv")
                        nc.vector.reciprocal(rv[:f_out, :], var[:f_out, :])
                        rstd = stpool.tile([f_out, 1], F32, tag="rstd", name="rstd")
                        nc.scalar.activation(rstd[:f_out, :], rv[:f_out, :], AF.Sqrt)
                        gs = stpool.tile([f_out, 1], F32, tag="gs", name="gs")
                        nc.vector.tensor_tensor(out=gs[:f_out, :], in0=g_sb[l][:f_out, :],
                                                in1=rstd[:f_out, :], op=ALU.mult)
                        cv = stpool.tile([f_out, 1], F32, tag="cv", name="cv")
                        nc.vector.tensor_tensor(out=cv[:f_out, :], in0=gs[:f_out, :],
                                                in1=mu[:f_out, :], op=ALU.mult)
                        nc.vector.tensor_tensor(out=cv[:f_out, :], in0=be_sb[l][:f_out, :],
                                                in1=cv[:f_out, :], op=ALU.subtract)
                        bn_cur = (gs, cv)
                    else:
                        bn_cur = None
                    prev_vT = vT

    nc.compile()
    return nc


_CACHE = {}


def _get_compiled(cfg, key, pre):
    if key not in _CACHE:
        _CACHE[key] = build_nc(cfg, pre["cts"])
    return _CACHE[key]


def _run(inputs, trace=False):
    cfg = Cfg(n_nodes=int(np.asarray(inputs["x"]).shape[0]))
    x = np.asarray(inputs["x"], dtype=np.float32)
    edge_index = np.asarray(inputs["edge_index"])
    pre = preprocess(cfg, x, edge_index)
    key = (cfg.n, edge_index.shape[1], hash(edge_index.tobytes()))
    nc = _get_compiled(cfg, key, pre)

    b6_rep = np.tile(np.asarray(inputs["b6"], dtype=np.float32)[None, :], (P, 1))
    bn_map = {0: "1", 1: "2", 3: "3", 4: "4"}
    in_maps = []
    for i in range(NCORES):
        m = {
            "xT": pre["xT"][i],
            "src_off": pre["src_off"][i],
            "dst_rel": pre["dst_rel"][i],
            "iota": pre["iota"],
            "ident": pre["ident"],
            "dis_col": pre["dis_col"][i],
            "dis_rep": pre["dis_rep"][i],
            "b6_rep": b6_rep,
        }
        for l in range(6):
            m[f"W{l}"] = np.asarray(inputs[f"W{l+1}"], dtype=np.float32)
            m[f"b{l}"] = np.asarray(inputs[f"b{l+1}"], dtype=np.float32)[:, None]
            if cfg.bn[l]:
                m[f"g{l}"] = np.asarray(inputs[f"g{bn_map[l]}"], dtype=np.float32)[:, None]
                m[f"be{l}"] = np.asarray(inputs[f"be{bn_map[l]}"], dtype=np.float32)[:, None]
        in_maps.append(m)

    res = run_bass_kernel_spmd(nc, in_maps, core_ids=list(range(NCORES)), trace=trace)
    if trace and res.instructions_and_trace is not None:
        print(f"TRACE PATH: {res.instructions_and_trace[1]}")
        print(f"PROFILE JSON: {res.profile_json}")
    parts = [res.results[i]["out"][:cfg.npc] for i in range(NCORES)]
    out = np.concatenate(parts, axis=0)
    return out, res.exec_time_ns


def kernel(**inputs) -> np.ndarray:
    out, _ = _run(inputs, trace=False)
    return out


def kernel_traced(**inputs):
    import trnprof  # noqa: F401  (registers the NTFF profile hook)
    return _run(inputs, trace=True)

